# revision 35
# baseline (speedup 1.0000x reference)
"""Block-local self-attention (BigBird-style window + one global token) on 8
Trainium2 NeuronCores.

Problem (hardcoded): n=2, h=16, t=4096, d=64, block=128, fp32 in/out,
attention_mask all-zeros.  Per (n,h) pair, query block g attends to K/V
positions [128(g-1), 128(g+2)) plus the global token 0; query 0 attends to all
4096 positions.

Sharding: pure data parallel - the 32 (n,h) pairs split 4 per core; no
collectives.

Device does ONLY the three big streams per pair:
  - QK: S^T per 128-token K-chunk j (K-chunk stationary, 384 attending
    queries moving) into [128, 2, 512] PSUM tiles, fp16.
  - exp on ACT per 2 chunks (768 cols amortizes the ACT access latency),
    fp16 out.  No masking: the kpos-0 "local copy" weight for query blocks
    0-1 equals the reference's global-column weight exp(q.K0), so it is kept.
  - AV out^T accumulated per 512-query PSUM bank: first writer start=True
    zeroes the whole 2KB bank (ZERO_REGION), the rest accumulate; V ships
    kpos-major with a ones column so Z rides row 64.  Eviction PSUM->SBUF
    fp16 alternates DVE/gpsimd, then one [65, 8*512] store per pair.
AV banks are woven into the same pair's QK group stream (bank b right after
its last needed chunk group) so the PE never idles between phases.

Host finishing (cheap, O(t) or O(t*d) numpy): adds the global-token rank-1
term e_g (x) [v0|1] for queries >= 256 (blocks 0-1 already got kpos 0 via
their window), normalizes by Z, computes the global-query row 0 exactly, and
transposes back to [t, d].
"""

import numpy as np

import concourse.bass as bass
import concourse.bacc as bacc
import concourse.tile as tile
from concourse import mybir
from concourse.bass_utils import run_bass_kernel_spmd

# ---- problem constants ----
N, H, T, D = 2, 16, 4096, 64
B = 128
NB = T // B            # 32 blocks
NAUG = D + 1           # V with ones column
NCORES = 8
NPAIR = (N * H) // NCORES   # 4 pairs per core
SCALE = 1.0 / np.sqrt(D)
BANKQ = 512            # query columns per out^T PSUM bank
NBANK = T // BANKQ     # 8

QK_DT = mybir.dt.float16
AV_DT = mybir.dt.float16
F32 = mybir.dt.float32


def _chunk_q0(j):
    return B * max(j - 1, 0)


def _chunk_q1(j):
    return min(B * (j + 2), T)


def _bank_writers():
    writers = [[] for _ in range(NBANK)]
    for j in range(NB):
        a, q1 = _chunk_q0(j), _chunk_q1(j)
        while a < q1:
            nxt = min(q1, (a // BANKQ + 1) * BANKQ)
            writers[a // BANKQ].append((j, a, nxt))
            a = nxt
    return writers


def build_nc(npair=NPAIR):
    nc = bacc.Bacc("TRN2", target_bir_lowering=False, debug=False)
    ncoup = npair // 2

    # qt is block-diagonal per couple: copy 0 = [Q_A^T; 0], copy 1 = [0; Q_B^T]
    # (partition-major layout [128, 2, T]).  The QK matmul then runs with
    # 128-partition moving data, which streams fp16 at 2 cols/cycle - twice
    # the rate of a 64-partition moving operand.  The couple-stacked kt
    # [K_A^T; K_B^T] is the shared 128-row stationary; the zero half of qt
    # kills the cross-pair terms.
    # copy-major so each [128, T] copy is a fully sequential DRAM read
    qt_d = nc.dram_tensor("qt", [ncoup, 2, 2 * D, T], QK_DT, kind="ExternalInput").ap()
    kt_d = nc.dram_tensor("kt", [ncoup, 2 * D, T], QK_DT, kind="ExternalInput").ap()
    # contiguous copy of couple 0's first FQ columns (kt row-stride in qt_d /
    # kt_d is 8-16KB, which makes the first strided loads crawl; this one
    # streams sequentially and lands in <1us)
    hd_d = nc.dram_tensor("hd", [2, 2 * D, 512], QK_DT, kind="ExternalInput").ap()
    va_d = nc.dram_tensor("va", [npair, B, NB * NAUG], AV_DT, kind="ExternalInput").ap()
    # unnormalized transposed output + Z row: [65, nbank, 512] fp16 per pair
    oz_d = nc.dram_tensor("oz", [npair, NAUG, NBANK * BANKQ], AV_DT,
                          kind="ExternalOutput").ap()

    Exp = mybir.ActivationFunctionType.Exp
    writers = _bank_writers()

    with tile.TileContext(nc) as tc:
        with (
            tc.tile_pool(name="qk", bufs=2) as qk_pool,
            tc.tile_pool(name="v", bufs=4) as v_pool,
            tc.tile_pool(name="e", bufs=2) as e_pool,
            tc.tile_pool(name="out", bufs=2) as out_pool,
            tc.tile_pool(name="qkps", bufs=2, space="PSUM") as qk_psum,
            tc.tile_pool(name="avps", bufs=4, space="PSUM") as av_psum,
        ):
            # ---- prologue: all input loads up front ----
            qts, kts, vas = [], [], []
            FQ = 512   # head segment: covers the first QK groups
            MQ = 1536
            for c in range(ncoup):
                qt_sb = qk_pool.tile([2 * D, 2, T], QK_DT, tag="qt")
                kt_sb = qk_pool.tile([2 * D, T], QK_DT, tag="kt")
                qts.append(qt_sb)
                kts.append(kt_sb)
            # warm the Exp activation table (~2.7us ACT_TABLE_LOAD) during
            # the input DMA wait instead of stalling the first real exp
            warm = e_pool.tile([1, 1], F32, tag="warm")
            nc.vector.memset(warm, 0.0)
            nc.scalar.activation(out=warm, in_=warm,
                                 func=Exp, scale=1.0)

            for ip in range(npair):
                va_sb = v_pool.tile([B, NB, NAUG], AV_DT, tag="va", name="va")
                vas.append(va_sb)
            # load order tuned so the first QK groups and first AV banks
            # never wait: couple-0 heads, couple-0 mids, couple-1 heads +
            # early va (on the otherwise-idle scalar queue), then the tails
            # heads from the contiguous copy (tiny, land <1us); everything
            # else as big near-sequential reads in first-need order
            nc.scalar.dma_start(out=qts[0][:, 0, 0:FQ], in_=hd_d[1])
            nc.sync.dma_start(out=kts[0][:, 0:FQ], in_=hd_d[0])
            nc.gpsimd.dma_start(out=qts[0][:, 0, FQ:T], in_=qt_d[0, 0, :, FQ:T])
            nc.sync.dma_start(out=kts[0][:, FQ:T], in_=kt_d[0, :, FQ:T])
            nc.scalar.dma_start(out=vas[0], in_=va_d[0])
            nc.gpsimd.dma_start(out=qts[0][:, 1, :], in_=qt_d[0, 1])
            nc.scalar.dma_start(out=vas[1], in_=va_d[1])
            nc.sync.dma_start(out=kts[1][:, 0:T], in_=kt_d[1])
            nc.gpsimd.dma_start(out=qts[1][:, 0, :], in_=qt_d[1, 0])
            nc.gpsimd.dma_start(out=qts[1][:, 1, :], in_=qt_d[1, 1])
            nc.gpsimd.dma_start(out=vas[2], in_=va_d[2])
            nc.sync.dma_start(out=vas[3], in_=va_d[3])

            exps = [None] * npair
            osbs = [None] * npair

            # ---------- per-pair unit streams ----------
            def qk_group(ip, g):
                c, hh = ip // 2, ip % 2
                qt_sb, kt_sb = qts[c], kts[c]

                def run():
                    if g == 0:
                        exps[ip] = e_pool.tile([B, NB, 3 * B], AV_DT, tag="exp",
                                               name="exp")
                        osbs[ip] = out_pool.tile([NAUG, NBANK, BANKQ], AV_DT,
                                                 tag="osb", name="osb")
                    ps = qk_psum.tile([B, 2, BANKQ], F32, tag="qkps")
                    for ti in range(2):
                        j = 2 * g + ti
                        # uniform 384-wide window (edge chunks widened so
                        # every exp call is one full batch)
                        q0w = min(_chunk_q0(j), T - 3 * B)
                        nc.tensor.matmul(
                            ps[:, ti, 0:3 * B],
                            lhsT=kt_sb[:, j * B:(j + 1) * B],
                            rhs=qt_sb[:, hh, q0w:q0w + 3 * B],
                            start=True,
                            stop=True,
                        )
                    nc.scalar.activation(
                        out=exps[ip][:, 2 * g:2 * g + 2, :],
                        in_=ps[:, :, 0:3 * B],
                        func=Exp, scale=float(SCALE),
                    )
                return run

            def av_banks(ip, blist):
                # emit the banks' writers interleaved: consecutive matmuls
                # then hit DIFFERENT psum banks, so the accumulator
                # read-modify-write of one bank overlaps the other's stream
                def run():
                    exp_sb = exps[ip]
                    va_sb = vas[ip]
                    avs, wls = [], []
                    for b in blist:
                        avs.append(av_psum.tile([NAUG, BANKQ], F32, tag="avps",
                                                name="avtile"))
                        wls.append(list(writers[b]))
                    for wi in range(max(len(w) for w in wls)):
                        for k, b in enumerate(blist):
                            if wi >= len(wls[k]):
                                continue
                            j, a0, a1 = wls[k][wi]
                            q0w = min(_chunk_q0(j), T - 3 * B)
                            nc.tensor.matmul(
                                avs[k][:, a0 - BANKQ * b:a1 - BANKQ * b],
                                lhsT=va_sb[:, j, :],
                                rhs=exp_sb[:, j, a0 - q0w:a1 - q0w],
                                start=(wi == 0),  # zeroes the whole 2KB bank
                                stop=(wi == len(wls[k]) - 1),
                                skip_group_check=(wi != 0),
                            )
                    for k, b in enumerate(blist):
                        # eviction on DVE (gpsimd cannot access PSUM), then a
                        # per-bank store so the output drains continuously
                        nc.vector.tensor_copy(out=osbs[ip][:, b, :], in_=avs[k])
                        seng = nc.sync if (ip * NBANK + b) % 2 == 0 else nc.gpsimd
                        seng.dma_start(
                            out=oz_d[ip, :, b * BANKQ:(b + 1) * BANKQ],
                            in_=osbs[ip][:, b, :],
                        )
                return run

            # ---------- emission: AV banks woven into the QK stream ----------
            # bank (p, b) consumes chunks up to 4b+6, i.e. QK group 2b+3 of
            # pair p.  Emit it SLACK groups later so the exp it needs is
            # already drained from ACT and the PE never stalls mid-stream;
            # late banks spill into the next pair's groups.
            # bank-pair (b0, b0+1) consumes chunks up to 4(b0+1)+6, i.e. QK
            # group 2*b0+5 of its pair.  The first pair runs with less slack
            # (no AV backlog exists yet to fill PE waits anyway) and the
            # last pair emits single banks as soon as their exps exist so
            # the tail after the final QK group is one bank, not four.
            NG = NB // 2
            av_ready = []
            for p in range(npair):
                slack = 1 if p == 0 else (3 if p < npair - 1 else 1)
                if p < npair - 1:
                    for b0 in range(0, NBANK, 2):
                        av_ready.append(
                            (NG * p + min(2 * b0 + 5, NG - 1) + slack,
                             p, (b0, b0 + 1)))
                else:
                    for b in range(NBANK):
                        av_ready.append(
                            (NG * p + min(2 * b + 3, NG - 1) + slack,
                             p, (b,)))
            av_ready.sort(key=lambda t: t[0])
            ai = 0
            for gi in range(npair * NG):
                qk_group(gi // NG, gi % NG)()
                while ai < len(av_ready) and av_ready[ai][0] <= gi:
                    _, p, blist = av_ready[ai]
                    av_banks(p, blist)()
                    ai += 1
            while ai < len(av_ready):
                _, p, blist = av_ready[ai]
                av_banks(p, blist)()
                ai += 1

    nc.compile()
    return nc


_CACHE = {}


def _prep_core(q, k, v, core):
    sl = slice(core * NPAIR, (core + 1) * NPAIR)
    np_qk = mybir.dt.np(QK_DT)
    qs, ks, vs = q[sl], k[sl], v[sl]
    ncoup = NPAIR // 2
    # kt: [ncoup, 2D, T] - two pairs of a couple stacked on partitions
    # qt: [ncoup, 2D, 2, T] block-diagonal: [:, 0:64, 0, :] = Q_A^T,
    #     [:, 64:128, 1, :] = Q_B^T, rest zeros
    qtt = qs.reshape(ncoup, 2, T, D).transpose(0, 1, 3, 2)  # [cp, 2, D, T]
    qt = np.zeros((ncoup, 2, 2, D, T), np.float32)  # [cp, copy, half, D, T]
    qt[:, 0, 0] = qtt[:, 0]
    qt[:, 1, 1] = qtt[:, 1]
    qt = np.ascontiguousarray(
        qt.reshape(ncoup, 2, 2 * D, T).astype(np_qk))
    kt = np.ascontiguousarray(
        ks.reshape(ncoup, 2, T, D).transpose(0, 1, 3, 2)
        .reshape(ncoup, 2 * D, T).astype(np_qk))
    # va: [npair, B, NB*NAUG] kpos-major with ones column
    va = np.concatenate([vs, np.ones((NPAIR, T, 1), np.float32)], axis=-1)
    va = va.reshape(NPAIR, NB, B, NAUG).transpose(0, 2, 1, 3)
    va = np.ascontiguousarray(
        va.reshape(NPAIR, B, NB * NAUG).astype(mybir.dt.np(AV_DT))
    )
    hd = np.ascontiguousarray(
        np.stack([kt[0, :, 0:512], qt[0, 0, :, 0:512]]))
    return {"qt": qt, "kt": kt, "va": va, "hd": hd}


def kernel(query_layer, key_layer, value_layer, attention_mask):
    q = np.asarray(query_layer, np.float32).reshape(N * H, T, D)
    k = np.asarray(key_layer, np.float32).reshape(N * H, T, D)
    v = np.asarray(value_layer, np.float32).reshape(N * H, T, D)

    if "nc" not in _CACHE:
        _CACHE["nc"] = build_nc()
    nc = _CACHE["nc"]

    in_maps = [_prep_core(q, k, v, core) for core in range(NCORES)]
    res = run_bass_kernel_spmd(nc, in_maps, core_ids=list(range(NCORES)))
    # [NCORES, NPAIR, 65, NBANK*BANKQ] fp16 -> [32, 65, 4096] f32
    oz = np.stack([r["oz"] for r in res.results]).astype(np.float32)
    oz = oz.reshape(N * H, NAUG, T)
    o_un = oz[:, 0:D, :]              # [32, 64, 4096] unnormalized out^T
    z = oz[:, D, :]                   # [32, 4096]

    # global-token rank-1 term for queries >= 2 blocks (blocks 0-1 already
    # include kpos 0 through their local window)
    eg = np.exp(np.einsum('ptd,pd->pt', q, k[:, 0]) * SCALE)  # [32, 4096]
    o_un[:, :, 2 * B:] += eg[:, None, 2 * B:] * v[:, 0, :, None]
    z[:, 2 * B:] += eg[:, 2 * B:]

    out = (o_un / z[:, None, :]).transpose(0, 2, 1)  # [32, 4096, 64]

    # global query row: exact softmax over all positions
    p0 = np.exp(np.einsum('pd,ptd->pt', q[:, 0], k) * SCALE)
    out[:, 0, :] = np.einsum('pt,ptd->pd', p0, v) / p0.sum(1)[:, None]

    return np.ascontiguousarray(out.reshape(N, H, T, D).astype(np.float32))
